# revision 40
# baseline (speedup 1.0000x reference)
"""Trainium2 Bass kernel for causal multi-head attention (B=2, T=2048, D=2048, H=16).

Sharding: head-tensor-parallel across 8 cores — core c computes heads {2c, 2c+1}
for both batches (QKV projections, scores, softmax, PV). The channel-major
attention outputs are then redistributed with an 8-rank AllToAll (each core
sends, per target j, its two heads' columns for output-row slice j), after
which core c holds ALL 2048 attention channels for 512 flattened (b,t) output
rows and computes those rows of the output projection with N=512 matmuls.
The A2A moves ~2MB/core (vs 16MB for an AllGather formulation) and is split
into two collectives (one per local head) so it overlaps compute.

Performance structure (from perfetto traces; 525/495us baseline -> ~416us):
- The PE streams bf16 matmuls at 1 column/cycle with LDWEIGHTS fully hidden,
  so phase 1 (QKV projections) is at the streaming roofline for the observed
  13/16 clock cap; its shape is kept. Startup is critical-path ordered: the
  first matmuls need wq rows 0-1 + xt rows 0-1, so those DMAs are queued
  first, finely interleaved, with wk slotted in just before the K matmuls
  need it (the old order hid the weight loads behind the full 4MB xt chunk,
  idling the PE until ~26us; now the first matmul issues at ~10us).
- Phase 2 processes query blocks in DESCENDING size order with double-buffered
  ov/dn PSUM so each block's softmax serial chain (exp -> mask -> pair-sum ->
  ones-matmul -> reciprocal -> broadcast -> normalize) hides under the next
  block's score matmuls. Scores are computed in pairs ([128,2,512] PSUM) with
  one exp() per pair and PV matmuls interleaved right after each pair's exp.
- Softmax denominators: DVE sums each exp'd pair, a second DVE add folds two
  pairs, and one ones-matmul per TWO pairs accumulates the column sums
  (quarter of the full denominator matmul streaming). The reciprocal uses the
  fast custom-DVE approximation (~5x faster than the exact [1,512] reciprocal,
  which is lane-serial and sat on the block critical path). The broadcast
  lands in a dn-pool PSUM slot (score pool stays a pure score-pair rotation)
  and its PSUM->SBUF stage runs on the Scalar engine (DVE is busier).
- The gpsimd queue carries ONLY the two collective triggers: it is a strict
  FIFO shared with them, and any compute queued there (measured with
  partition_broadcast and tensor_add variants) stalls an A2A by tens of us.
  Similarly, Wo is loaded as one early 8MB transfer at the start of phase 2:
  a Wo chunk DMA'd concurrently with an A2A stretched it from ~21 to ~42us.
- The last block before each A2A trigger has nothing to hide its softmax
  chain under (it gates the trigger): those two "flush" blocks skip the DVE
  folds (dn matmuls read the exp'd scores directly) and pre-stage ov in
  SBUF so the final multiply reads the broadcast PSUM without a staging
  copy — the flush chain drops from ~6.8us to ~5.5us.
- Phase 3 runs in two passes: the ct-blocks from A2A#1 are contracted into
  PSUM (during A2A#2's flight) and drained to SBUF as f32 partials; the
  second pass contracts A2A#2's blocks and a DVE add merges the partials
  into bf16 (halves the output DMA on the kernel tail).
- fp8 was evaluated and rejected: e4m3 QKV projections give 6e-2 rel err
  (gate 2e-2). N=1024 matmul dsts fail the ISA check (PSUM dst is one
  512-f32 bank); fp32 matmuls run at 4 cycles/row — both avoided.
"""

import numpy as np
import ml_dtypes

import concourse.bass as bass
import concourse.bacc as bacc
import concourse.mybir as mybir
import concourse.tile as tile
from concourse.bass_utils import run_bass_kernel_spmd

B, T, D, H, HD = 2, 2048, 2048, 16, 128
NCORES = 8
HPC = H // NCORES        # heads per core = 2
CW = HPC * HD            # channel slice per core = 256
NDT = D // 128           # 16 contraction tiles
NTQ = T // 512           # 4 query blocks of 512
NTK = T // 128           # 16 key tiles of 128
RPC = (B * T) // NCORES  # flattened output rows per core = 512
SCALE = 1.0 / float(np.sqrt(HD))

BF16 = mybir.dt.bfloat16
F32 = mybir.dt.float32
BF = ml_dtypes.bfloat16

_CACHE = {}

# lane order: head-local 0 of both batches first, then head-local 1 — so the
# first A2A (channels = every rank's head0) can fire halfway through phase 2.
LANES = [(0, 0), (1, 0), (0, 1), (1, 1)]  # (b, hl)


def _emit_rep(nc, tc, dram, params, rep):
    xt_p = params["xt"]          # [B][D, T] bf16 (query transposed)
    masks_sb = params["masks_sb"]
    wq_sb, wk_sb, wv_sb = params["wq_sb"], params["wk_sb"], params["wv_sb"]
    ones_col = params["ones_col"]
    ones_row = params["ones_row"]
    qh_sb, kh_sb, v_sb = params["qh_sb"], params["kh_sb"], params["v_sb"]
    out_p = params["out"]

    # DRAM staging for the two AllToAlls (one per local head).
    # cc_in[l] shard j (128 rows) = at((b(j), l), tqb=j%4)  [128ch, 512tq]
    cc_in = [dram.tile([NCORES * HD, 512], BF16, name=f"cc_in{l}_{rep}")
             for l in range(HPC)]
    cc_out = [dram.tile([NCORES * HD, 512], BF16, name=f"cc_out{l}_{rep}")
              for l in range(HPC)]

    wvs = {w: params[w][:].rearrange("(n p) j -> p n j", p=128)
           for w in ("wq", "wk", "wv")}

    # ---- Phase 1: QKV projections (per batch, per tq-half) ----
    with tc.tile_pool(name="p1", bufs=1) as p1, \
         tc.tile_pool(name="psum1", bufs=1, space="PSUM") as psum1:
        for b in range(B):
            xv = xt_p[b][:].rearrange("(n p) t -> p n t", p=128)
            for th in range(2):  # tq half
                xt_sb = p1.tile([128, NDT, T // 2], BF16, tag="xt", bufs=2,
                                name="xt_sb")

                def _xt_chunk(ch, th=th, xv=xv, xt_sb=xt_sb):
                    nc.sync.dma_start(
                        out=xt_sb[:, 4 * ch:4 * ch + 4, :],
                        in_=xv[:, 4 * ch:4 * ch + 4,
                               th * (T // 2):(th + 1) * (T // 2)])

                def _w_chunk(w, ch):
                    nc.sync.dma_start(
                        out=params[w + "_sb"][:, 4 * ch:4 * ch + 4, :],
                        in_=wvs[w][:, 4 * ch:4 * ch + 4, :])

                if b == 0 and th == 0 and rep == 0:
                    # startup-critical order: the first matmuls consume
                    # wq dt-rows 0.. + xt dt-rows 0..; queue those first,
                    # finely interleaved so the PE can start ~2 dt-rows in.
                    for ch in range(8):
                        nc.sync.dma_start(
                            out=wq_sb[:, 2 * ch:2 * ch + 2, :],
                            in_=wvs["wq"][:, 2 * ch:2 * ch + 2, :])
                        nc.sync.dma_start(
                            out=xt_sb[:, 2 * ch:2 * ch + 2, :],
                            in_=xv[:, 2 * ch:2 * ch + 2, 0:T // 2])
                        if ch >= 4:  # wk rows land just before K's matmuls
                            _w_chunk("wk", ch - 4)
                    for ch in range(4):
                        _w_chunk("wv", ch)
                else:
                    for ch in range(4):
                        _xt_chunk(ch)

                # Q.T / K.T per head: [hd=128, tq]
                for hl in range(HPC):
                    lane = 2 * b + hl
                    for w_sb, dst in ((wq_sb, qh_sb), (wk_sb, kh_sb)):
                        ps = psum1.tile([128, 2, 512], F32, tag="qk", bufs=2,
                                        name="ps_qk")
                        for dt in range(NDT):
                            for tq in range(2):
                                nc.tensor.matmul(
                                    ps[:, tq, :],
                                    lhsT=w_sb[:, dt, hl * 128:(hl + 1) * 128],
                                    rhs=xt_sb[:, dt, tq * 512:(tq + 1) * 512],
                                    start=(dt == 0), stop=(dt == NDT - 1))
                        tqg = th * 1024
                        nc.vector.tensor_copy(
                            dst[:, lane, tqg:tqg + 1024],
                            ps[:].rearrange("p i j -> p (i j)"))
                # V in natural layout [tk, ch]
                for tkt in range(NTK // 2):
                    tkg = th * (NTK // 2) + tkt
                    ps = psum1.tile([128, CW], F32, tag="vproj", bufs=3,
                                    name="ps_v")
                    for dt in range(NDT):
                        nc.tensor.matmul(
                            ps[:],
                            lhsT=xt_sb[:, dt, tkt * 128:(tkt + 1) * 128],
                            rhs=wv_sb[:, dt, :],
                            start=(dt == 0), stop=(dt == NDT - 1))
                    nc.vector.tensor_copy(v_sb[:, b * NTK + tkg, :], ps[:])

    # ---- Phase 2: attention (lane order: head0 lanes first) + A2As ----
    # The GPSIMD queue carries ONLY the two collective triggers (the softmax
    # broadcast is a K=1 PE matmul) so the Tile scheduler cannot delay them.
    wop_cm = tc.tile_pool(name="wop", bufs=1)
    wop = wop_cm.__enter__()
    wo_sb = wop.tile([128, NDT, D], BF16, name="wo_sb")
    with tc.tile_pool(name="p2", bufs=1) as p2, \
         tc.tile_pool(name="psum2", bufs=1, space="PSUM") as psum2:
        if rep == 0:
            # masks + the full Wo now — after phase-1 query DMAs but well
            # before the A2As (a concurrent 4MB Wo load was measured to
            # stretch the A2A from ~21us to ~42us).
            nc.sync.dma_start(out=masks_sb[:],
                              in_=params["masks"][:].rearrange(
                                  "i p j -> p i j"))
            wov = params["wo"][:].rearrange("(n p) j -> p n j", p=128)
            for ch in range(8):
                nc.sync.dma_start(out=wo_sb[:, 2 * ch:2 * ch + 2, :],
                                  in_=wov[:, 2 * ch:2 * ch + 2, :])
        # Deferred-work queue: PV/denominator/normalize for each score pair
        # is emitted ~2 pairs after its scores, so the PE never waits on the
        # exp -> mask -> sum chain of the pair it just scored. Closures cross
        # query-block and lane boundaries uniformly.
        dq = []

        def _drain(nleft):
            while len(dq) > nleft:
                dq.pop(0)()

        for li, (b, hl) in enumerate(LANES):
            lane = 2 * b + hl
            if li == 2:
                # lanes (b,h0) both done and their finishers flushed:
                # cc_in[0] is complete -> first A2A
                _drain(0)
                nc.gpsimd.collective_compute(
                    "AllToAll", mybir.AluOpType.bypass,
                    replica_groups=[list(range(NCORES))],
                    ins=[cc_in[0][:]], outs=[cc_out[0][:]])
            for tqb in reversed(range(NTQ)):  # descending block size
                nkt = 4 * (tqb + 1)
                npair = nkt // 2
                pt = p2.tile([128, NTK, 512], BF16, tag="pt", bufs=2,
                             name="pt")
                dn = psum2.tile([128, 512], F32, tag="dn", bufs=2, name="dn")
                ov = psum2.tile([128, 512], F32, tag="ov", bufs=2, name="ov")
                sps = []   # per-pair sums (indexed by pr)
                s4s = []   # 2-pair folds (for the 4-pair dn fold)
                # the last block before each A2A trigger is fully serial
                # (nothing left to hide it under): shorten its chain by
                # skipping the DVE folds (dn direct from pt) and pre-staging
                # ov in SBUF so _norm can read the broadcast PSUM directly.
                flush = b == 1 and tqb == 0
                ov_sb = [None]

                def _pv_dn(pr, pt=pt, dn=dn, ov=ov, sps=sps, s4s=s4s, b=b,
                           hl=hl, nkt=nkt, npair=npair, flush=flush,
                           ov_sb=ov_sb):
                    # PV + denominator contribution of pair pr. The pair
                    # sums fold 2x (and 4x where the block allows) on the
                    # DVE so the dn matmul streams 1/4 of the exp'd scores:
                    # phase 2 is Tensor-bound, DVE has slack.
                    k0 = 2 * pr
                    for i in range(2):
                        kt = k0 + i
                        nc.tensor.matmul(
                            ov[:],
                            lhsT=v_sb[:, b * NTK + kt,
                                      hl * 128:(hl + 1) * 128],
                            rhs=pt[:, kt, :],
                            start=(kt == 0), stop=(kt == nkt - 1))
                    if flush:
                        for i in range(2):
                            kt = k0 + i
                            nc.tensor.matmul(
                                dn[0:1, :], lhsT=ones_col[:],
                                rhs=pt[:, kt, :],
                                start=(kt == 0), stop=(kt == nkt - 1))
                        if pr == npair - 1:
                            osb = p2.tile([128, 512], BF16, tag="ov_sb",
                                          bufs=2, name="ov_sb")
                            nc.vector.tensor_copy(osb[:], ov[:])
                            ov_sb[0] = osb
                        return
                    sp = p2.tile([128, 512], BF16, tag="s2", bufs=6,
                                 name="s2")
                    nc.vector.tensor_add(sp[:], pt[:, k0, :],
                                         pt[:, k0 + 1, :])
                    sps.append(sp)
                    if pr % 2 != 1:
                        return
                    s4 = p2.tile([128, 512], BF16, tag="s4", bufs=3,
                                 name="s4")
                    nc.vector.tensor_add(s4[:], sps[pr - 1][:], sps[pr][:])
                    s4s.append(s4)
                    first = npair < 4 and pr == 1
                    if pr % 4 == 3:  # fold two 2-pair sums
                        s8 = p2.tile([128, 512], BF16, tag="s4", bufs=3,
                                     name="s4")
                        nc.vector.tensor_add(s8[:], s4s[pr // 2 - 1][:],
                                             s4[:])
                        rhs, first = s8, (pr == 3)
                    elif pr == npair - 1:  # trailing 2-pair fold
                        rhs = s4
                    else:
                        return
                    nc.tensor.matmul(
                        dn[0:1, :], lhsT=ones_col[:], rhs=rhs[:],
                        start=first, stop=(pr == npair - 1))

                def _norm(dn=dn, ov=ov, b=b, tqb=tqb, hl=hl, flush=flush,
                          ov_sb=ov_sb):
                    # softmax normalize + ship to the A2A staging buffer.
                    # NOTE: the gpsimd queue must stay empty (strict FIFO
                    # shared with the collective triggers — anything queued
                    # there stalls an A2A by tens of us), so the broadcast
                    # is a K=1 PE matmul.
                    rc = p2.tile([1, 512], F32, tag="rc", bufs=2, name="rc")
                    nc.vector.reciprocal_approx_fast(out=rc[:],
                                                     in_=dn[0:1, :])
                    rcb = p2.tile([1, 512], BF16, tag="rcb", bufs=2,
                                  name="rcb")
                    with nc.allow_low_precision(reason="softmax denom bf16"):
                        nc.vector.tensor_copy(rcb[:], rc[:])
                    # broadcast via K=1 matmul into a dn-pool PSUM slot (the
                    # score pool is the scarcer resource: keeping it for
                    # score pairs only removes a per-block stall)
                    bcs = psum2.tile([128, 512], F32, tag="dn", bufs=2,
                                     name="dn")
                    bc = bcs[:]
                    nc.tensor.matmul(bc, lhsT=ones_row[:], rhs=rcb[:],
                                     start=True, stop=True)
                    at = p2.tile([128, 512], BF16, tag="at", bufs=3,
                                 name="at")
                    if flush:
                        # ov was pre-staged in SBUF: multiply against the
                        # broadcast PSUM directly (one PSUM operand)
                        nc.vector.tensor_mul(at[:], bc, ov_sb[0][:])
                    else:
                        # DVE can read only one PSUM operand: stage bc in
                        # SBUF (on the scalar engine — DVE is busier)
                        bc_sb = p2.tile([128, 512], BF16, tag="bc_sb",
                                        bufs=2, name="bc_sb")
                        nc.scalar.activation(
                            bc_sb[:], bc,
                            mybir.ActivationFunctionType.Copy)
                        nc.vector.tensor_mul(at[:], ov[:], bc_sb[:])
                    j = 4 * b + tqb
                    nc.sync.dma_start(
                        out=cc_in[hl][j * 128:(j + 1) * 128, :], in_=at[:])

                for pr in range(npair):
                    k0 = 2 * pr
                    ps = psum2.tile([128, 2, 512], F32, tag="score", bufs=2,
                                    name="ps_s")
                    for i in range(2):
                        kt = k0 + i
                        nc.tensor.matmul(
                            ps[:, i, :],
                            lhsT=kh_sb[:, lane, kt * 128:(kt + 1) * 128],
                            rhs=qh_sb[:, lane, tqb * 512:(tqb + 1) * 512],
                            start=True, stop=True)
                    _drain(2)
                    nc.scalar.activation(
                        pt[:, k0:k0 + 2, :], ps[:],
                        mybir.ActivationFunctionType.Exp, scale=SCALE)
                    if k0 >= 4 * tqb:  # diagonal-block pair: apply mask
                        mi = k0 - 4 * tqb
                        nc.vector.tensor_mul(
                            pt[:, k0:k0 + 2, :].rearrange("p i j -> p (i j)"),
                            pt[:, k0:k0 + 2, :].rearrange("p i j -> p (i j)"),
                            masks_sb[:, mi:mi + 2, :].rearrange(
                                "p i j -> p (i j)"))
                    dq.append(lambda pr=pr, pv=_pv_dn: pv(pr))
                dq.append(_norm)
        _drain(0)
        nc.gpsimd.collective_compute(
            "AllToAll", mybir.AluOpType.bypass,
            replica_groups=[list(range(NCORES))],
            ins=[cc_in[1][:]], outs=[cc_out[1][:]])

    # ---- Phase 3: output projection for my 512 flattened rows ----
    # A_all channel blocks: pass l=0 -> global heads 0,2,..,14 (A2A#1),
    # pass l=1 -> heads 1,3,..,15 (A2A#2). wo rows host-permuted to match;
    # pass l streams wo chunk l ([128, 8, 2048] = dt rows 8l..8l+7).
    with tc.tile_pool(name="p3", bufs=1) as p3, \
         tc.tile_pool(name="psum3", bufs=1, space="PSUM") as psum3:
        a_sb = [None, None]
        part = [[None] * 2 for _ in range(4)]  # [tqt][cwh] f32 partials
        for l in range(HPC):
            a_sb[l] = p3.tile([128, NCORES, 512], BF16, name=f"a_sb{l}")
            cv = cc_out[l][:].rearrange("(ct p) t -> p ct t", p=128)
            for i in range(NCORES):
                nc.sync.dma_start(out=a_sb[l][:, i, :], in_=cv[:, i, :])
            for tqt in range(4):
                for cwh in range(2):
                    po = psum3.tile([128, 2, 512], F32, tag="po", bufs=4,
                                    name="po")
                    for ct in range(NCORES):
                        for cwg in range(2):
                            cw0 = cwh * 1024 + cwg * 512
                            nc.tensor.matmul(
                                po[:, cwg, :],
                                lhsT=a_sb[l][:, ct,
                                             tqt * 128:(tqt + 1) * 128],
                                rhs=wo_sb[:, l * NCORES + ct,
                                          cw0:cw0 + 512],
                                start=(ct == 0), stop=(ct == NCORES - 1))
                    if l == 0:
                        pp = p3.tile([128, 2, 512], F32,
                                     tag=f"part{tqt}_{cwh}", bufs=1,
                                     name=f"part{tqt}_{cwh}")
                        part[tqt][cwh] = pp
                        nc.vector.tensor_copy(pp[:], po[:])
                    else:
                        ot = p3.tile([128, 2, 512], BF16, tag="ot", bufs=3,
                                     name="ot")
                        with nc.allow_low_precision(reason="bf16 output"):
                            nc.vector.tensor_add(ot[:], po[:],
                                                 part[tqt][cwh][:])
                        nc.sync.dma_start(
                            out=out_p[tqt * 128:(tqt + 1) * 128,
                                      cwh * 1024:(cwh + 1) * 1024],
                            in_=ot[:].rearrange("p i j -> p (i j)"))
    wop_cm.__exit__(None, None, None)


def _build(reps: int = 1):
    nc = bacc.Bacc("TRN2", target_bir_lowering=False, debug=False,
                   num_devices=NCORES)

    params = {}
    params["xt"] = [nc.declare_dram_parameter(f"xt{b}", [D, T], BF16,
                                              isOutput=False)
                    for b in range(B)]
    for w in ("wq", "wk", "wv"):
        params[w] = nc.declare_dram_parameter(w, [D, CW], BF16,
                                              isOutput=False)
    params["wo"] = nc.declare_dram_parameter("wo", [D, D], BF16,
                                             isOutput=False)
    params["masks"] = nc.declare_dram_parameter("masks", [4, 128, 512], BF16,
                                                isOutput=False)
    params["out"] = nc.declare_dram_parameter("out", [RPC, D], BF16,
                                              isOutput=True)

    with tile.TileContext(nc) as tc:
        with tc.tile_pool(name="consts", bufs=1) as consts, \
             tc.tile_pool(name="qkv", bufs=1) as qkv, \
             tc.tile_pool(name="dram", bufs=1, space="DRAM") as dram:

            for w, nm in (("wq", "wq_sb"), ("wk", "wk_sb"), ("wv", "wv_sb")):
                params[nm] = consts.tile([128, NDT, CW], BF16, name=nm)
            masks_sb = consts.tile([128, 4, 512], BF16, name="masks_sb")
            params["masks_sb"] = masks_sb
            ones_col = consts.tile([128, 1], BF16, name="ones_col")
            nc.vector.memset(ones_col[:], 1.0)
            params["ones_col"] = ones_col
            ones_row = consts.tile([1, 128], BF16, name="ones_row")
            nc.vector.memset(ones_row[:], 1.0)
            params["ones_row"] = ones_row

            # channel-major Q.T/K.T + natural V, resident through phase 2
            params["qh_sb"] = qkv.tile([128, B * HPC, T], BF16, name="qh_sb")
            params["kh_sb"] = qkv.tile([128, B * HPC, T], BF16, name="kh_sb")
            params["v_sb"] = qkv.tile([128, B * NTK, CW], BF16, name="v_sb")

            for rep in range(reps):
                _emit_rep(nc, tc, dram, params, rep)

    nc.compile()
    return nc


def _get_nc(reps: int = 1):
    key = f"nc{reps}"
    if key not in _CACHE:
        _CACHE[key] = _build(reps)
    return _CACHE[key]


def make_in_maps(query, Wq, Wk, Wv, Wo):
    """Per-core input maps (shared host-side prep for kernel() and test)."""
    query = np.asarray(query, dtype=np.float32)
    xt = [np.ascontiguousarray(query[b].T).astype(BF) for b in range(B)]
    p_idx = np.arange(128)[:, None]
    j_idx = np.arange(512)[None, :]
    masks = np.stack([(p_idx <= j_idx - 128 * i) for i in range(4)]
                     ).astype(BF)
    # Wo rows (input channels) permuted to the A2A channel-block order:
    # heads [0,2,...,14, 1,3,...,15]
    woT = np.ascontiguousarray(np.asarray(Wo, np.float32).T)
    perm = [h for h in range(0, H, 2)] + [h for h in range(1, H, 2)]
    wo_perm = np.concatenate([woT[128 * h:128 * (h + 1)] for h in perm],
                             axis=0).astype(BF)
    in_maps = []
    for c in range(NCORES):
        sl = slice(CW * c, CW * (c + 1))
        in_maps.append({
            "xt0": xt[0],
            "xt1": xt[1],
            "wq": np.ascontiguousarray(np.asarray(Wq, np.float32)[sl].T
                                       ).astype(BF),
            "wk": np.ascontiguousarray(np.asarray(Wk, np.float32)[sl].T
                                       ).astype(BF),
            "wv": np.ascontiguousarray(np.asarray(Wv, np.float32)[sl].T
                                       ).astype(BF),
            "wo": wo_perm,
            "masks": masks,
        })
    return in_maps


def kernel(query, attention_mask, Wq, Wk, Wv, Wo, bo):
    nc = _get_nc()
    in_maps = make_in_maps(query, Wq, Wk, Wv, Wo)
    res = run_bass_kernel_spmd(nc, in_maps, list(range(NCORES))).results
    flat = np.concatenate([np.asarray(res[c]["out"], dtype=np.float32)
                           for c in range(NCORES)], axis=0)
    out = flat.reshape(B, T, D) + np.asarray(bo, np.float32)[None, None, :]
    return out.astype(np.float32)
